# revision 36
# baseline (speedup 1.0000x reference)
"""Fused transformer block (pre-norm attn + MLP) for Trainium2, 8 cores.

Sharding: data-parallel over batch (32 batches -> 4 per core), no
collectives. Each core computes the full block on its shard.

v3 design notes (vs v2 at 672us; this version ~532us):
- LN rsqrt via DVE reciprocal_approx_fast + one ACT Sqrt instead of
  Ln+Exp: Ln/Exp live in different ACT table-sets, so v2 paid 2 table
  loads (~2.6us each) per LN chunk (31 loads total; now ~10).
- P1 processes the token stream in global chunks (128,384,512x3,260)
  instead of per-batch (512,65): the 65-wide qkv matmuls were
  LDWEIGHTS-bound (107ns for 27ns of stream). LN1 output h lives in
  persistent [128, T] tiles so per-batch v k-tiles can slice any token
  range regardless of chunk boundaries. qkv weights stream in 3 column
  groups (q, k, v) at P1 top so the first qkv matmuls start early.
  All qkv psum evacs ride ACT Identity (ACT is ~20% busy in P1; the
  LN chains keep DVE at ~65%, and evacs on DVE stalled the psum ring).
- P2 exp runs as 2 ACT instructions per head instead of 5: scores for
  k-tiles 0-3 x queries 0-511 land in one [128, 2048] psum quad (4
  banks), everything else (k-tile 4 + the 65-query tails of all 5
  k-tiles) in one [128, 837] tile. ACT overhead is ~352 cols per
  instruction, so 5 -> 2 instructions saves ~0.9us/head (~42us).
  The kt4 stationary is padded to 128 columns (qk_sb over-allocated by
  64 junk cols, memset 0) so the B-tile psum is fully written and the
  single exp never reads unwritten psum.
- q zero-pad staging buffers are filled by SBUF->SBUF DMA (idle engine)
  instead of ACT Identity (v2: 48 x 640ns on the P2-critical ACT).
- softmax denominators: per-head [1,577] reciprocal_approx_fast (DVE
  custom op, ~5x faster than the iterative reciprocal), result already
  on partition 0 so gpsimd partition_broadcast needs no staging copies.
  The o-normalize multiply trails its head by 3 iterations so the
  gpsimd broadcast latency hides. (Measured regressions to avoid:
  gpsimd general tensor ops ~2x-10x slower than DVE; psum-tag sharing
  between PE streams and DVE-gated slots; ACT-side evacs; f32 oh with
  direct-recip -> NaN. Keep mult/den/evac on DVE, pools separate.)
- emission order per head: front (scores+exps) BEFORE the previous
  head's attn@v, so the PE never head-of-line blocks on ACT exp.
- proj is emitted one n-tile per head during the NEXT batch's first 6
  heads; psum stays at 8 banks (quad 4 + B 2 + av-q0 1 + small-shared 1
  for av-tail/proj tiles).
LN gains/biases and the attention scale are folded into the weights on
the host. All matmul operands bf16; psum f32; MLP weights fp8+DoubleRow.
ACT exp has no max subtraction (|scores| < 3).
"""
import numpy as np
import ml_dtypes
import concourse.bacc as bacc
import concourse.mybir as mybir
import concourse.tile as tile
from concourse.bass_utils import run_bass_kernel_spmd

F32 = mybir.dt.float32
BF16 = mybir.dt.bfloat16
F8 = mybir.dt.float8e4
DR = mybir.MatmulPerfMode.DoubleRow
WS = 16.0  # fp8 weight scale (w*WS stored fp8; ACT evac rescales by 1/WS)
AF = mybir.ActivationFunctionType
ALU = mybir.AluOpType

B, N, C = 32, 577, 768
H, D = 12, 64
HID = 3072
NCORES = 8
BPC = B // NCORES            # 4 batches per core
T = BPC * N                  # 2308 tokens per core
TP = T + 64                  # qk_sb padded so kt4 stationaries are 128 wide
CHUNKS = [(0, 512), (512, 512), (1024, 512), (1536, 512), (2048, 260)]
LCH = [(0, 256), (256, 256), (512, 512), (1024, 512), (1536, 512), (2048, 260)]
KTILES = [(0, 128), (128, 128), (256, 128), (384, 128), (512, 65)]
# v k-tiles flushed after the LN chunk that completes them:
# (batch, i, abs_start, rows) grouped by first chunk index that covers them
VFLUSH = {ci: [] for ci in range(len(LCH))}
for _b in range(BPC):
    for _i, (_k0, _kr) in enumerate(KTILES):
        _end = _b * N + _k0 + _kr
        for _ci, (_c0, _cw) in enumerate(LCH):
            if _end <= _c0 + _cw:
                VFLUSH[_ci].append((_b, _i, _b * N + _k0, _kr))
                break


def _build_nc():
    nc = bacc.Bacc("TRN2", target_bir_lowering=False, debug=False,
                   num_devices=NCORES)
    xT_d = nc.dram_tensor("xT", [C, T], BF16, kind="ExternalInput")
    wqkv_d = nc.dram_tensor("wqkv", [C, 3 * C], BF16, kind="ExternalInput")
    qkb_d = nc.dram_tensor("qkb", [128, 12], F32, kind="ExternalInput")
    vbb_d = nc.dram_tensor("vbb", [128, C], F32, kind="ExternalInput")
    wp_d = nc.dram_tensor("wp", [6, 128, C], BF16, kind="ExternalInput")
    pb_d = nc.dram_tensor("pb", [128, 6], F32, kind="ExternalInput")
    w1_d = nc.dram_tensor("w1", [3, 128, 2 * HID], F8, kind="ExternalInput")
    b1a_d = nc.dram_tensor("b1a", [128, 24], F32, kind="ExternalInput")
    w2_d = nc.dram_tensor("w2", [12, 128, 2 * C], F8, kind="ExternalInput")
    b2a_d = nc.dram_tensor("b2a", [128, 6], F32, kind="ExternalInput")
    outT_d = nc.dram_tensor("outT", [C, T], F32, kind="ExternalOutput")

    with tile.TileContext(nc) as tc:
        with tc.tile_pool(name="cst", bufs=1) as cst, \
             tc.tile_pool(name="x2p", bufs=1) as x2p:
            ones128 = cst.tile([128, 128], BF16)
            nc.vector.memset(ones128[:], 1.0)
            qkb = cst.tile([128, 12], F32)
            nc.sync.dma_start(out=qkb[:], in_=qkb_d[:])
            vbb = cst.tile([128, C], F32)
            nc.sync.dma_start(out=vbb[:], in_=vbb_d[:])
            pb = cst.tile([128, 6], F32)
            nc.sync.dma_start(out=pb[:], in_=pb_d[:])
            b1a = cst.tile([128, 24], F32)
            nc.sync.dma_start(out=b1a[:], in_=b1a_d[:])
            b2a = cst.tile([128, 6], F32)
            nc.sync.dma_start(out=b2a[:], in_=b2a_d[:])
            x2 = [x2p.tile([128, T], BF16, name=f"x2_{k}") for k in range(6)]

            with tc.tile_pool(name="qks", bufs=1) as qks, \
                 tc.tile_pool(name="vbp", bufs=1) as vbp:
                qk_sb = [qks.tile([128, TP], BF16, name=f"qk{n}")
                         for n in range(12)]
                for n in range(6, 12):
                    nc.vector.memset(qk_sb[n][:, T:TP], 0.0)
                vbuf = {}
                for b in range(BPC):
                    for i in range(5):
                        vbuf[(b, i)] = vbp.tile([128, H * 65], BF16,
                                                name=f"vb{b}_{i}")
                        ocol = vbuf[(b, i)].rearrange(
                            "p (h e) -> p h e", e=65)[:, :, 64]
                        nc.vector.memset(ocol, 1.0)

                # ---------------- P1: LN1 + qk + v ----------------
                with tc.tile_pool(name="p1w", bufs=1) as p1w, \
                     tc.tile_pool(name="p1hh", bufs=1) as p1hh:
                    wq = [p1w.tile([128, 3 * C], BF16, name=f"wq{k}")
                          for k in range(6)]
                    h_all = [p1hh.tile([128, T], BF16, name=f"ha{k}")
                             for k in range(6)]
                    with tc.tile_pool(name="p1x", bufs=2) as p1x, \
                         tc.tile_pool(name="p1s", bufs=2) as p1s, \
                         tc.tile_pool(name="ps1", bufs=1, space="PSUM") as ps1, \
                         tc.tile_pool(name="psqk", bufs=2, space="PSUM") as psqk, \
                         tc.tile_pool(name="psv", bufs=2, space="PSUM") as psv:
                        xc = {}

                        def emit_stats(ci):
                            c0, cw = LCH[ci]
                            xs = []
                            for k in range(6):
                                xt = p1x.tile([128, cw], BF16,
                                              name=f"x{k}_{c0}", tag=f"x{k}")
                                nc.sync.dma_start(
                                    out=xt[:],
                                    in_=xT_d[k * 128:(k + 1) * 128,
                                             c0:c0 + cw])
                                xs.append(xt)
                            xc[ci] = xs
                            ps_sum = ps1.tile([128, cw], F32,
                                              name=f"pss_{c0}", tag="ps_sum")
                            ps_ssq = ps1.tile([128, cw], F32,
                                              name=f"psq_{c0}", tag="ps_ssq")
                            for k in range(6):
                                xq = p1x.tile([128, cw], BF16,
                                              name=f"xq{k}_{c0}", tag="xq")
                                nc.vector.tensor_tensor(xq[:], xs[k][:],
                                                        xs[k][:], ALU.mult)
                                nc.tensor.matmul(ps_sum[:], ones128[:],
                                                 xs[k][:],
                                                 start=(k == 0), stop=(k == 5))
                                nc.tensor.matmul(ps_ssq[:], ones128[:], xq[:],
                                                 start=(k == 0), stop=(k == 5))
                            return ps_sum, ps_ssq

                        def emit_ln(ci, ps_sum, ps_ssq):
                            c0, cw = LCH[ci]
                            mu = p1s.tile([128, cw], F32, name=f"mu_{c0}",
                                          tag="mu")
                            nc.vector.tensor_scalar(mu[:], ps_sum[:], 1.0 / C,
                                                    None, ALU.mult)
                            t1 = p1s.tile([128, cw], F32, name=f"t1_{c0}",
                                          tag="tt")
                            nc.vector.tensor_tensor(t1[:], ps_sum[:], mu[:],
                                                    ALU.mult)
                            t2 = p1s.tile([128, cw], F32, name=f"t2_{c0}",
                                          tag="tt")
                            nc.vector.tensor_tensor(t2[:], ps_ssq[:], t1[:],
                                                    ALU.subtract)
                            t3 = p1s.tile([128, cw], F32, name=f"t3_{c0}",
                                          tag="tt")
                            nc.vector.tensor_scalar(t3[:], t2[:], 1.0 / C,
                                                    1e-5, ALU.mult, ALU.add)
                            iv = p1s.tile([128, cw], F32, name=f"iv_{c0}",
                                          tag="iv")
                            nc.vector.reciprocal_approx_fast(iv[:], t3[:])
                            rs = p1s.tile([128, cw], BF16, name=f"rs_{c0}",
                                          tag="rs")
                            nc.scalar.activation(rs[:], iv[:], AF.Sqrt)
                            murs = p1s.tile([128, cw], BF16, name=f"mr_{c0}",
                                            tag="mr")
                            nc.vector.tensor_tensor(murs[:], mu[:], rs[:],
                                                    ALU.mult)
                            for k in range(6):
                                tmp = p1s.tile([128, cw], F32,
                                               name=f"tp{k}_{c0}", tag="tp")
                                nc.vector.tensor_tensor(tmp[:], xc[ci][k][:],
                                                        rs[:], ALU.mult)
                                nc.vector.tensor_tensor(
                                    h_all[k][:, c0:c0 + cw], tmp[:], murs[:],
                                    ALU.subtract)
                            del xc[ci]

                        def emit_qkv(ci):
                            c0, cw = LCH[ci]
                            for n in range(12):
                                pq = psqk.tile([128, cw], F32,
                                               name=f"pq{n}_{c0}", tag="psqk")
                                for k in range(6):
                                    nc.tensor.matmul(
                                        pq[:], wq[k][:, n * 128:(n + 1) * 128],
                                        h_all[k][:, c0:c0 + cw],
                                        start=(k == 0), stop=(k == 5))
                                nc.scalar.activation(
                                    qk_sb[n][:, c0:c0 + cw], pq[:],
                                    AF.Identity, bias=qkb[:, n:n + 1])

                        def emit_v(ci):
                            for (b, i, a0, kr) in VFLUSH[ci]:
                                pv = psv.tile([128, C], F32,
                                              name=f"pv_{b}_{i}", tag="psv")
                                for k in range(6):
                                    nc.tensor.matmul(
                                        pv[:kr, 0:512],
                                        h_all[k][:, a0:a0 + kr],
                                        wq[k][:, 1536:2048],
                                        start=(k == 0), stop=(k == 5))
                                for k in range(6):
                                    nc.tensor.matmul(
                                        pv[:kr, 512:768],
                                        h_all[k][:, a0:a0 + kr],
                                        wq[k][:, 2048:2304],
                                        start=(k == 0), stop=(k == 5))
                                vm = vbuf[(b, i)]
                                dst = vm[0:kr].rearrange(
                                    "p (h e) -> p h e", e=65)[:, :, 0:64]
                                srcv = pv[0:kr].rearrange(
                                    "p (h e) -> p h e", e=64)
                                bia = vbb[0:kr].rearrange(
                                    "p (h e) -> p h e", e=64)
                                nc.vector.tensor_tensor(dst, srcv, bia,
                                                        ALU.add)

                        for g0, g1 in ((0, 768), (768, 1536), (1536, 2304)):
                            for k in range(6):
                                nc.scalar.dma_start(
                                    out=wq[k][:, g0:g1],
                                    in_=wqkv_d[k * 128:(k + 1) * 128, g0:g1])
                        emit_ln(0, *emit_stats(0))
                        emit_ln(1, *emit_stats(1))
                        for ci in range(len(LCH)):
                            emit_qkv(ci)
                            emit_v(ci)
                            if ci + 2 < len(LCH):
                                emit_ln(ci + 2, *emit_stats(ci + 2))

                # ---------------- P2: attention + proj ----------------
                with tc.tile_pool(name="wpp", bufs=1) as wpp, \
                     tc.tile_pool(name="qbp", bufs=1) as qbp, \
                     tc.tile_pool(name="oal", bufs=2) as oal, \
                     tc.tile_pool(name="p2t", bufs=2) as p2t, \
                     tc.tile_pool(name="pexp", bufs=2) as pexp, \
                     tc.tile_pool(name="psA", bufs=1, space="PSUM") as psA, \
                     tc.tile_pool(name="psB", bufs=1, space="PSUM") as psB, \
                     tc.tile_pool(name="psC", bufs=1, space="PSUM") as psC, \
                     tc.tile_pool(name="psS", bufs=1, space="PSUM") as psS:
                    wp_sb = [wpp.tile([128, C], BF16, name=f"wp{j}")
                             for j in range(6)]
                    qbbuf = [qbp.tile([128, N], BF16, name=f"qb{j}")
                             for j in range(4)]
                    for j in range(4):
                        p0 = (j % 2) * 64
                        nc.vector.memset(qbbuf[j][64 - p0:128 - p0, :], 0.0)

                    seq = [(b, h) for b in range(BPC) for h in range(H)]
                    o_tiles = {}
                    estate = {}

                    def qb_dma(idx):
                        b, h = seq[idx]
                        base = b * N
                        p0 = (h % 2) * 64
                        nc.sync.dma_start(
                            out=qbbuf[h % 4][p0:p0 + 64, :],
                            in_=qk_sb[h // 2][p0:p0 + 64, base:base + N])

                    def front(idx):
                        b, h = seq[idx]
                        if idx + 2 < len(seq):
                            qb_dma(idx + 2)
                        base = b * N
                        qb = qbbuf[h % 4]
                        kt = qk_sb[6 + h // 2]
                        quad = psA.tile([128, 2048], F32, name=f"qd{b}_{h}",
                                        tag="quad")
                        for i in range(4):
                            nc.tensor.matmul(
                                quad[:, i * 512:(i + 1) * 512],
                                kt[:, base + i * 128:base + (i + 1) * 128],
                                qb[:, 0:512], start=True, stop=True)
                        eA = pexp.tile([128, 2048], BF16, name=f"eA{b}_{h}",
                                       tag="eA")
                        nc.scalar.activation(eA[:], quad[:], AF.Exp)
                        bt = psB.tile([128, 837], F32, name=f"bt{b}_{h}",
                                      tag="sb")
                        nc.tensor.matmul(bt[:, 0:512],
                                         kt[:, base + 512:base + 640],
                                         qb[:, 0:512], start=True, stop=True)
                        for i in range(5):
                            ks = kt[:, base + i * 128:base + i * 128 + 128]
                            nc.tensor.matmul(
                                bt[:, 512 + i * 65:512 + (i + 1) * 65],
                                ks, qb[:, 512:577], start=True, stop=True)
                        eB = pexp.tile([128, 837], BF16, name=f"eB{b}_{h}",
                                       tag="eB")
                        nc.scalar.activation(eB[:], bt[:], AF.Exp)
                        estate[idx] = (eA, eB)

                    def mid(idx):
                        b, h = seq[idx]
                        eA, eB = estate.pop(idx)
                        pso = psC.tile([65, 512], F32, name=f"po{b}_{h}",
                                       tag="pso")
                        for i in range(5):
                            kr = KTILES[i][1]
                            mov = (eA[:, i * 512:(i + 1) * 512] if i < 4
                                   else eB[:, 0:512])
                            nc.tensor.matmul(pso[:],
                                             vbuf[(b, i)][:kr,
                                                          h * 65:(h + 1) * 65],
                                             mov[:kr, :],
                                             start=(i == 0), stop=(i == 4))
                        avt = psS.tile([65, 65], F32, name=f"pt{b}_{h}",
                                       tag="sm")
                        for i in range(5):
                            kr = KTILES[i][1]
                            nc.tensor.matmul(
                                avt[:],
                                vbuf[(b, i)][:kr, h * 65:(h + 1) * 65],
                                eB[:kr, 512 + i * 65:512 + (i + 1) * 65],
                                start=(i == 0), stop=(i == 4))
                        oh = p2t.tile([65, N], BF16, name=f"oh{b}_{h}",
                                      tag=f"oh{h % 2}")
                        nc.vector.tensor_copy(oh[:, 0:512], pso[:])
                        nc.vector.tensor_copy(oh[:, 512:577], avt[:])
                        den = p2t.tile([1, N], F32, name=f"dn{b}_{h}",
                                       tag="den")
                        nc.vector.tensor_copy(den[:], oh[64:65, :])
                        rcp = p2t.tile([1, N], F32, name=f"rc{b}_{h}",
                                       tag="rcp")
                        nc.vector.reciprocal_approx_fast(rcp[:], den[:])
                        bc = p2t.tile([64, N], F32, name=f"bc{b}_{h}",
                                      tag=f"bc{h % 2}")
                        nc.gpsimd.partition_broadcast(bc[:], rcp[0:1, :])
                        return oh, bc

                    ostate = {}

                    def tail(idx):
                        b, h = seq[idx]
                        oh, bc = ostate.pop(idx)
                        p0 = (h % 2) * 64
                        nc.vector.tensor_tensor(
                            o_tiles[b][h // 2][p0:p0 + 64, :],
                            oh[0:64, :], bc[:], ALU.mult)

                    def emit_proj_n(b, n):
                        base = b * N
                        ppq = psS.tile([128, 512], F32, name=f"pp{b}_{n}",
                                       tag="sm")
                        ppt = psS.tile([128, 65], F32, name=f"pq{b}_{n}",
                                       tag="sm")
                        for j in range(6):
                            nc.tensor.matmul(
                                ppq[:], wp_sb[j][:, n * 128:(n + 1) * 128],
                                o_tiles[b][j][:, 0:512],
                                start=(j == 0), stop=(j == 5))
                        for j in range(6):
                            nc.tensor.matmul(
                                ppt[:], wp_sb[j][:, n * 128:(n + 1) * 128],
                                o_tiles[b][j][:, 512:577],
                                start=(j == 0), stop=(j == 5))
                        xr = p2t.tile([128, N], BF16, name=f"xr{b}_{n}",
                                      tag=f"xr{n % 2}")
                        nc.sync.dma_start(
                            out=xr[:],
                            in_=xT_d[n * 128:(n + 1) * 128, base:base + N])
                        nc.vector.scalar_tensor_tensor(
                            x2[n][:, base:base + 512], ppq[:],
                            pb[:, n:n + 1], xr[:, 0:512], ALU.add, ALU.add)
                        nc.vector.scalar_tensor_tensor(
                            x2[n][:, base + 512:base + N], ppt[:],
                            pb[:, n:n + 1], xr[:, 512:577], ALU.add, ALU.add)

                    qb_dma(0)
                    qb_dma(1)
                    for j in range(6):
                        nc.scalar.dma_start(out=wp_sb[j][:], in_=wp_d[j])
                    for idx, (b, h) in enumerate(seq):
                        if h == 0:
                            o_tiles[b] = [oal.tile([128, N], BF16,
                                                   name=f"oa{b}_{j}",
                                                   tag=f"oa{j}")
                                          for j in range(6)]
                        front(idx)
                        if idx >= 1:
                            ostate[idx - 1] = mid(idx - 1)
                        if idx >= 3:
                            tail(idx - 3)
                        if b > 0 and 2 <= h < 8:
                            emit_proj_n(b - 1, h - 2)
                    ostate[len(seq) - 1] = mid(len(seq) - 1)
                    tail(len(seq) - 3)
                    tail(len(seq) - 2)
                    tail(len(seq) - 1)
                    for n in range(6):
                        emit_proj_n(BPC - 1, n)

            # ---------------- P4: LN2 + MLP ----------------
            with tc.tile_pool(name="w12", bufs=1) as w12, \
                 tc.tile_pool(name="p4x", bufs=2) as p4x, \
                 tc.tile_pool(name="p4s", bufs=2) as p4s, \
                 tc.tile_pool(name="h2p", bufs=2) as h2p, \
                 tc.tile_pool(name="p4t", bufs=2) as p4t, \
                 tc.tile_pool(name="pgl", bufs=2) as pgl, \
                 tc.tile_pool(name="ps4s", bufs=1, space="PSUM") as ps4s, \
                 tc.tile_pool(name="ps41", bufs=4, space="PSUM") as ps41, \
                 tc.tile_pool(name="ps42", bufs=2, space="PSUM") as ps42:
                w1_sb = [w12.tile([128, 2 * HID], F8, name=f"w1_{k}")
                         for k in range(3)]
                for k in range(3):
                    nc.sync.dma_start(out=w1_sb[k][:], in_=w1_d[k])
                w2_sb = [w12.tile([128, 2 * C], F8, name=f"w2_{k}")
                         for k in range(12)]
                h2c = {}

                def emit_stats2(ci):
                    c0, cw = CHUNKS[ci]
                    ps_sum = ps4s.tile([128, cw], F32, name=f"2ss_{c0}",
                                       tag="ps_sum2")
                    ps_ssq = ps4s.tile([128, cw], F32, name=f"2sq_{c0}",
                                       tag="ps_ssq2")
                    for k in range(6):
                        xq = p4x.tile([128, cw], BF16, name=f"2xq{k}_{c0}",
                                      tag=f"2xq{k}")
                        nc.vector.tensor_tensor(xq[:], x2[k][:, c0:c0 + cw],
                                                x2[k][:, c0:c0 + cw], ALU.mult)
                        nc.tensor.matmul(ps_sum[:], ones128[:],
                                         x2[k][:, c0:c0 + cw],
                                         start=(k == 0), stop=(k == 5))
                        nc.tensor.matmul(ps_ssq[:], ones128[:], xq[:],
                                         start=(k == 0), stop=(k == 5))
                    mu = p4s.tile([128, cw], F32, name=f"2mu_{c0}", tag="2mu")
                    nc.vector.tensor_scalar(mu[:], ps_sum[:], 1.0 / C, None,
                                            ALU.mult)
                    t1 = p4s.tile([128, cw], F32, name=f"2t1_{c0}", tag="2tt")
                    nc.vector.tensor_tensor(t1[:], ps_sum[:], mu[:], ALU.mult)
                    t2 = p4s.tile([128, cw], F32, name=f"2t2_{c0}", tag="2tt")
                    nc.vector.tensor_tensor(t2[:], ps_ssq[:], t1[:],
                                            ALU.subtract)
                    t3 = p4s.tile([128, cw], F32, name=f"2t3_{c0}", tag="2tt")
                    nc.vector.tensor_scalar(t3[:], t2[:], 1.0 / C, 1e-5,
                                            ALU.mult, ALU.add)
                    iv = p4s.tile([128, cw], F32, name=f"2iv_{c0}", tag="2iv")
                    nc.vector.reciprocal_approx_fast(iv[:], t3[:])
                    rs = p4s.tile([128, cw], BF16, name=f"2rs_{c0}", tag="2rs")
                    nc.scalar.activation(rs[:], iv[:], AF.Sqrt)
                    murs = p4s.tile([128, cw], BF16, name=f"2mr_{c0}",
                                    tag="2mr")
                    nc.vector.tensor_tensor(murs[:], mu[:], rs[:], ALU.mult)
                    hs = []
                    for i in range(3):
                        hp = h2p.tile([128, 2 * cw], F8, name=f"2h{i}_{c0}",
                                      tag=f"2h{i}")
                        hs.append(hp)
                    for k in range(6):
                        tmp = p4s.tile([128, cw], F32, name=f"2tp{k}_{c0}",
                                       tag="2tp")
                        nc.vector.tensor_tensor(tmp[:], x2[k][:, c0:c0 + cw],
                                                rs[:], ALU.mult)
                        dst = hs[k // 2][:, (k % 2) * cw:(k % 2) * cw + cw]
                        with nc.allow_low_precision(reason="fp8 mlp act"):
                            nc.vector.tensor_tensor(dst, tmp[:], murs[:],
                                                    ALU.subtract)
                    h2c[ci] = hs

                P4CH = [4, 0, 1, 2, 3]
                emit_stats2(P4CH[0])
                for pi in range(5):
                    ci = P4CH[pi]
                    c0, cw = CHUNKS[ci]
                    hs = h2c.pop(ci)
                    gl = [pgl.tile([128, 2 * cw], F8, name=f"gl{j}_{c0}",
                                   tag=f"gl{j}") for j in range(12)]
                    for n1 in range(24):
                        p1p = ps41.tile([128, cw], F32, name=f"p41_{n1}_{c0}",
                                        tag="ps41")
                        for i in range(3):
                            w3 = w1_sb[i].rearrange(
                                "p (s n) -> p s n", s=2)[
                                :, :, n1 * 128:(n1 + 1) * 128]
                            h3 = hs[i].rearrange("p (s f) -> p s f", s=2)
                            nc.tensor.matmul(p1p[:], w3, h3,
                                             start=(i == 0), stop=(i == 2),
                                             perf_mode=DR)
                        gdst = gl[n1 // 2][:, (n1 % 2) * cw:(n1 % 2) * cw + cw]
                        with nc.allow_low_precision(reason="fp8 mlp act"):
                            nc.scalar.activation(gdst, p1p[:], AF.Gelu,
                                                 bias=b1a[:, n1:n1 + 1],
                                                 scale=1.0 / WS)
                    if pi == 0:
                        for k in range(12):
                            nc.sync.dma_start(out=w2_sb[k][:], in_=w2_d[k])
                    if pi + 1 < 5:
                        emit_stats2(P4CH[pi + 1])
                    for n2 in range(6):
                        p2p = ps42.tile([128, cw], F32, name=f"p42_{n2}_{c0}",
                                        tag="ps42")
                        for j in range(12):
                            w3 = w2_sb[j].rearrange(
                                "p (s n) -> p s n", s=2)[
                                :, :, n2 * 128:(n2 + 1) * 128]
                            g3 = gl[j].rearrange("p (s f) -> p s f", s=2)
                            nc.tensor.matmul(p2p[:], w3, g3,
                                             start=(j == 0), stop=(j == 11),
                                             perf_mode=DR)
                        t2o = p4t.tile([128, cw], F32, name=f"t2o{n2}_{c0}",
                                       tag="t2o")
                        nc.scalar.activation(t2o[:], p2p[:], AF.Identity,
                                             bias=b2a[:, n2:n2 + 1],
                                             scale=1.0 / WS)
                        oo = p4t.tile([128, cw], F32, name=f"oo{n2}_{c0}",
                                      tag="oo")
                        nc.vector.tensor_tensor(oo[:], t2o[:],
                                                x2[n2][:, c0:c0 + cw], ALU.add)
                        nc.sync.dma_start(
                            out=outT_d[n2 * 128:(n2 + 1) * 128, c0:c0 + cw],
                            in_=oo[:])
    nc.compile()
    return nc


_CACHE = {}


def _prep_shared(inputs):
    f32 = np.float32
    bf = ml_dtypes.bfloat16
    qkv_w = np.asarray(inputs["qkv_w"], f32)
    ln1_g = np.asarray(inputs["ln1_g"], f32)
    ln1_b = np.asarray(inputs["ln1_b"], f32)
    qkv_b = np.asarray(inputs["qkv_b"], f32)
    W = qkv_w * ln1_g[:, None]
    bq = ln1_b @ qkv_w + qkv_b
    W = W.copy()
    W[:, :C] *= 0.125
    bq = bq.copy()
    bq[:C] *= 0.125

    proj_w = np.asarray(inputs["proj_w"], f32)
    fc1_w = np.asarray(inputs["fc1_w"], f32)
    ln2_g = np.asarray(inputs["ln2_g"], f32)
    ln2_b = np.asarray(inputs["ln2_b"], f32)
    fc1_b = np.asarray(inputs["fc1_b"], f32)
    W1 = fc1_w * ln2_g[:, None]
    b1 = ln2_b @ fc1_w + fc1_b
    fc2_w = np.asarray(inputs["fc2_w"], f32)

    f8 = ml_dtypes.float8_e4m3
    w1s = (W1 * WS).astype(f8).reshape(3, 2, 128, HID)
    w1s = w1s.transpose(0, 2, 1, 3).reshape(3, 128, 2 * HID)
    w2s = (fc2_w * WS).astype(f8).reshape(12, 2, 128, C)
    w2s = w2s.transpose(0, 2, 1, 3).reshape(12, 128, 2 * C)
    return {
        "wqkv": np.ascontiguousarray(W.astype(bf)),
        "qkb": np.ascontiguousarray(bq[:2 * C].reshape(12, 128).T.astype(f32)),
        "vbb": np.ascontiguousarray(np.tile(bq[2 * C:], (128, 1)).astype(f32)),
        "wp": np.ascontiguousarray(proj_w.reshape(6, 128, C).astype(bf)),
        "pb": np.ascontiguousarray(
            np.asarray(inputs["proj_b"], f32).reshape(6, 128).T),
        "w1": np.ascontiguousarray(w1s),
        "b1a": np.ascontiguousarray(b1.reshape(24, 128).T.astype(f32)),
        "w2": np.ascontiguousarray(w2s),
        "b2a": np.ascontiguousarray(
            np.asarray(inputs["fc2_b"], f32).reshape(6, 128).T),
    }


def _make_in_maps(inputs):
    bf = ml_dtypes.bfloat16
    x = np.asarray(inputs["x"], np.float32)
    shared = _prep_shared(inputs)
    in_maps = []
    for c in range(NCORES):
        xT = np.ascontiguousarray(
            x[c * BPC:(c + 1) * BPC].reshape(T, C).T.astype(bf))
        m = {"xT": xT}
        m.update(shared)
        in_maps.append(m)
    return in_maps


def kernel(**inputs):
    if "nc" not in _CACHE:
        _CACHE["nc"] = _build_nc()
    nc = _CACHE["nc"]
    in_maps = _make_in_maps(inputs)
    res = run_bass_kernel_spmd(nc, in_maps, list(range(NCORES)))
    out = np.empty((B, N, C), np.float32)
    for c in range(NCORES):
        outT = res.results[c]["outT"]
        out[c * BPC:(c + 1) * BPC] = outT.T.reshape(BPC, N, C)
    return out


# revision 37
# speedup vs baseline: 1.0257x; 1.0257x over previous
"""Fused transformer block (pre-norm attn + MLP) for Trainium2, 8 cores.

Sharding: data-parallel over batch (32 batches -> 4 per core), no
collectives. Each core computes the full block on its shard.

v3 design notes (vs v2 at 672us; this version ~532us):
- LN rsqrt via DVE reciprocal_approx_fast + one ACT Sqrt instead of
  Ln+Exp: Ln/Exp live in different ACT table-sets, so v2 paid 2 table
  loads (~2.6us each) per LN chunk (31 loads total; now ~10).
- P1 processes the token stream in global chunks (128,384,512x3,260)
  instead of per-batch (512,65): the 65-wide qkv matmuls were
  LDWEIGHTS-bound (107ns for 27ns of stream). LN1 output h lives in
  persistent [128, T] tiles so per-batch v k-tiles can slice any token
  range regardless of chunk boundaries. qkv weights stream in 3 column
  groups (q, k, v) at P1 top so the first qkv matmuls start early.
  All qkv psum evacs ride ACT Identity (ACT is ~20% busy in P1; the
  LN chains keep DVE at ~65%, and evacs on DVE stalled the psum ring).
- P2 exp runs as 2 ACT instructions per head instead of 5: scores for
  k-tiles 0-3 x queries 0-511 land in one [128, 2048] psum quad (4
  banks), everything else (k-tile 4 + the 65-query tails of all 5
  k-tiles) in one [128, 837] tile. ACT overhead is ~352 cols per
  instruction, so 5 -> 2 instructions saves ~0.9us/head (~42us).
  The kt4 stationary is padded to 128 columns (qk_sb over-allocated by
  64 junk cols, memset 0) so the B-tile psum is fully written and the
  single exp never reads unwritten psum.
- q zero-pad staging buffers are filled by SBUF->SBUF DMA (idle engine)
  instead of ACT Identity (v2: 48 x 640ns on the P2-critical ACT).
- softmax denominators: per-head [1,577] reciprocal_approx_fast (DVE
  custom op, ~5x faster than the iterative reciprocal), result already
  on partition 0 so gpsimd partition_broadcast needs no staging copies.
  The o-normalize multiply trails its head by 3 iterations so the
  gpsimd broadcast latency hides. (Measured regressions to avoid:
  gpsimd general tensor ops ~2x-10x slower than DVE; psum-tag sharing
  between PE streams and DVE-gated slots; ACT-side evacs; f32 oh with
  direct-recip -> NaN. Keep mult/den/evac on DVE, pools separate.)
- emission order per head: front (scores+exps) BEFORE the previous
  head's attn@v, so the PE never head-of-line blocks on ACT exp.
- proj is emitted one n-tile per head during the NEXT batch's first 6
  heads; psum stays at 8 banks (quad 4 + B 2 + av-q0 1 + small-shared 1
  for av-tail/proj tiles).
LN gains/biases and the attention scale are folded into the weights on
the host. All matmul operands bf16; psum f32; MLP weights fp8+DoubleRow.
ACT exp has no max subtraction (|scores| < 3).
"""
import numpy as np
import ml_dtypes
import concourse.bacc as bacc
import concourse.mybir as mybir
import concourse.tile as tile
from concourse.bass_utils import run_bass_kernel_spmd

F32 = mybir.dt.float32
BF16 = mybir.dt.bfloat16
F8 = mybir.dt.float8e4
DR = mybir.MatmulPerfMode.DoubleRow
WS = 16.0  # fp8 weight scale (w*WS stored fp8; ACT evac rescales by 1/WS)
AF = mybir.ActivationFunctionType
ALU = mybir.AluOpType

B, N, C = 32, 577, 768
H, D = 12, 64
HID = 3072
NCORES = 8
BPC = B // NCORES            # 4 batches per core
T = BPC * N                  # 2308 tokens per core
TP = T + 64                  # qk_sb padded so kt4 stationaries are 128 wide
CHUNKS = [(0, 512), (512, 512), (1024, 512), (1536, 512), (2048, 260)]
LCH = [(0, 256), (256, 256), (512, 512), (1024, 512), (1536, 512), (2048, 260)]
KTILES = [(0, 128), (128, 128), (256, 128), (384, 128), (512, 65)]
# v k-tiles flushed after the LN chunk that completes them:
# (batch, i, abs_start, rows) grouped by first chunk index that covers them
VFLUSH = {ci: [] for ci in range(len(LCH))}
for _b in range(BPC):
    for _i, (_k0, _kr) in enumerate(KTILES):
        _end = _b * N + _k0 + _kr
        for _ci, (_c0, _cw) in enumerate(LCH):
            if _end <= _c0 + _cw:
                VFLUSH[_ci].append((_b, _i, _b * N + _k0, _kr))
                break


def _build_nc():
    nc = bacc.Bacc("TRN2", target_bir_lowering=False, debug=False,
                   num_devices=NCORES)
    xT_d = nc.dram_tensor("xT", [C, T], BF16, kind="ExternalInput")
    wqkv_d = nc.dram_tensor("wqkv", [C, 3 * C], BF16, kind="ExternalInput")
    qkb_d = nc.dram_tensor("qkb", [128, 12], F32, kind="ExternalInput")
    vbb_d = nc.dram_tensor("vbb", [128, C], F32, kind="ExternalInput")
    wp_d = nc.dram_tensor("wp", [6, 128, C], BF16, kind="ExternalInput")
    pb_d = nc.dram_tensor("pb", [128, 6], F32, kind="ExternalInput")
    w1_d = nc.dram_tensor("w1", [3, 128, 2 * HID], F8, kind="ExternalInput")
    b1a_d = nc.dram_tensor("b1a", [128, 24], F32, kind="ExternalInput")
    w2_d = nc.dram_tensor("w2", [12, 128, 2 * C], F8, kind="ExternalInput")
    b2a_d = nc.dram_tensor("b2a", [128, 6], F32, kind="ExternalInput")
    outT_d = nc.dram_tensor("outT", [C, T], F32, kind="ExternalOutput")

    with tile.TileContext(nc) as tc:
        with tc.tile_pool(name="cst", bufs=1) as cst, \
             tc.tile_pool(name="x2p", bufs=1) as x2p:
            ones128 = cst.tile([128, 128], BF16)
            nc.vector.memset(ones128[:], 1.0)
            qkb = cst.tile([128, 12], F32)
            nc.sync.dma_start(out=qkb[:], in_=qkb_d[:])
            vbb = cst.tile([128, C], F32)
            nc.sync.dma_start(out=vbb[:], in_=vbb_d[:])
            pb = cst.tile([128, 6], F32)
            nc.sync.dma_start(out=pb[:], in_=pb_d[:])
            b1a = cst.tile([128, 24], F32)
            nc.sync.dma_start(out=b1a[:], in_=b1a_d[:])
            b2a = cst.tile([128, 6], F32)
            nc.sync.dma_start(out=b2a[:], in_=b2a_d[:])
            x2 = [x2p.tile([128, T], BF16, name=f"x2_{k}") for k in range(6)]

            with tc.tile_pool(name="qks", bufs=1) as qks, \
                 tc.tile_pool(name="vbp", bufs=1) as vbp:
                qk_sb = [qks.tile([128, TP], BF16, name=f"qk{n}")
                         for n in range(12)]
                for n in range(6, 12):
                    nc.vector.memset(qk_sb[n][:, T:TP], 0.0)
                vbuf = {}
                for b in range(BPC):
                    for i in range(5):
                        vbuf[(b, i)] = vbp.tile([128, H * 65], BF16,
                                                name=f"vb{b}_{i}")
                        ocol = vbuf[(b, i)].rearrange(
                            "p (h e) -> p h e", e=65)[:, :, 64]
                        nc.vector.memset(ocol, 1.0)

                # ---------------- P1: LN1 + qk + v ----------------
                with tc.tile_pool(name="p1w", bufs=1) as p1w, \
                     tc.tile_pool(name="p1hh", bufs=1) as p1hh:
                    wq = [p1w.tile([128, 3 * C], BF16, name=f"wq{k}")
                          for k in range(6)]
                    h_all = [p1hh.tile([128, T], BF16, name=f"ha{k}")
                             for k in range(6)]
                    with tc.tile_pool(name="p1x", bufs=2) as p1x, \
                         tc.tile_pool(name="p1s", bufs=2) as p1s, \
                         tc.tile_pool(name="ps1", bufs=1, space="PSUM") as ps1, \
                         tc.tile_pool(name="psqk", bufs=2, space="PSUM") as psqk, \
                         tc.tile_pool(name="psv", bufs=2, space="PSUM") as psv:
                        xc = {}

                        def emit_stats(ci):
                            c0, cw = LCH[ci]
                            xs = []
                            for k in range(6):
                                xt = p1x.tile([128, cw], BF16,
                                              name=f"x{k}_{c0}", tag=f"x{k}")
                                nc.sync.dma_start(
                                    out=xt[:],
                                    in_=xT_d[k * 128:(k + 1) * 128,
                                             c0:c0 + cw])
                                xs.append(xt)
                            xc[ci] = xs
                            ps_sum = ps1.tile([128, cw], F32,
                                              name=f"pss_{c0}", tag="ps_sum")
                            ps_ssq = ps1.tile([128, cw], F32,
                                              name=f"psq_{c0}", tag="ps_ssq")
                            for k in range(6):
                                xq = p1x.tile([128, cw], BF16,
                                              name=f"xq{k}_{c0}", tag="xq")
                                nc.vector.tensor_tensor(xq[:], xs[k][:],
                                                        xs[k][:], ALU.mult)
                                nc.tensor.matmul(ps_sum[:], ones128[:],
                                                 xs[k][:],
                                                 start=(k == 0), stop=(k == 5))
                                nc.tensor.matmul(ps_ssq[:], ones128[:], xq[:],
                                                 start=(k == 0), stop=(k == 5))
                            return ps_sum, ps_ssq

                        def emit_ln(ci, ps_sum, ps_ssq):
                            c0, cw = LCH[ci]
                            mu = p1s.tile([128, cw], F32, name=f"mu_{c0}",
                                          tag="mu")
                            nc.vector.tensor_scalar(mu[:], ps_sum[:], 1.0 / C,
                                                    None, ALU.mult)
                            t1 = p1s.tile([128, cw], F32, name=f"t1_{c0}",
                                          tag="tt")
                            nc.vector.tensor_tensor(t1[:], ps_sum[:], mu[:],
                                                    ALU.mult)
                            t2 = p1s.tile([128, cw], F32, name=f"t2_{c0}",
                                          tag="tt")
                            nc.vector.tensor_tensor(t2[:], ps_ssq[:], t1[:],
                                                    ALU.subtract)
                            t3 = p1s.tile([128, cw], F32, name=f"t3_{c0}",
                                          tag="tt")
                            nc.vector.tensor_scalar(t3[:], t2[:], 1.0 / C,
                                                    1e-5, ALU.mult, ALU.add)
                            iv = p1s.tile([128, cw], F32, name=f"iv_{c0}",
                                          tag="iv")
                            nc.vector.reciprocal_approx_fast(iv[:], t3[:])
                            rs = p1s.tile([128, cw], BF16, name=f"rs_{c0}",
                                          tag="rs")
                            nc.scalar.activation(rs[:], iv[:], AF.Sqrt)
                            murs = p1s.tile([128, cw], BF16, name=f"mr_{c0}",
                                            tag="mr")
                            nc.vector.tensor_tensor(murs[:], mu[:], rs[:],
                                                    ALU.mult)
                            for k in range(6):
                                tmp = p1s.tile([128, cw], F32,
                                               name=f"tp{k}_{c0}", tag="tp")
                                nc.vector.tensor_tensor(tmp[:], xc[ci][k][:],
                                                        rs[:], ALU.mult)
                                nc.vector.tensor_tensor(
                                    h_all[k][:, c0:c0 + cw], tmp[:], murs[:],
                                    ALU.subtract)
                            del xc[ci]

                        def emit_qkv(ci):
                            c0, cw = LCH[ci]
                            for n in range(12):
                                pq = psqk.tile([128, cw], F32,
                                               name=f"pq{n}_{c0}", tag="psqk")
                                for k in range(6):
                                    nc.tensor.matmul(
                                        pq[:], wq[k][:, n * 128:(n + 1) * 128],
                                        h_all[k][:, c0:c0 + cw],
                                        start=(k == 0), stop=(k == 5))
                                nc.scalar.activation(
                                    qk_sb[n][:, c0:c0 + cw], pq[:],
                                    AF.Identity, bias=qkb[:, n:n + 1])

                        def emit_v(ci):
                            for (b, i, a0, kr) in VFLUSH[ci]:
                                pv = psv.tile([128, C], F32,
                                              name=f"pv_{b}_{i}", tag="psv")
                                for k in range(6):
                                    nc.tensor.matmul(
                                        pv[:kr, 0:512],
                                        h_all[k][:, a0:a0 + kr],
                                        wq[k][:, 1536:2048],
                                        start=(k == 0), stop=(k == 5))
                                for k in range(6):
                                    nc.tensor.matmul(
                                        pv[:kr, 512:768],
                                        h_all[k][:, a0:a0 + kr],
                                        wq[k][:, 2048:2304],
                                        start=(k == 0), stop=(k == 5))
                                vm = vbuf[(b, i)]
                                dst = vm[0:kr].rearrange(
                                    "p (h e) -> p h e", e=65)[:, :, 0:64]
                                srcv = pv[0:kr].rearrange(
                                    "p (h e) -> p h e", e=64)
                                bia = vbb[0:kr].rearrange(
                                    "p (h e) -> p h e", e=64)
                                nc.vector.tensor_tensor(dst, srcv, bia,
                                                        ALU.add)

                        for g0, g1 in ((0, 768), (768, 1536), (1536, 2304)):
                            for k in range(6):
                                nc.scalar.dma_start(
                                    out=wq[k][:, g0:g1],
                                    in_=wqkv_d[k * 128:(k + 1) * 128, g0:g1])
                        emit_ln(0, *emit_stats(0))
                        emit_ln(1, *emit_stats(1))
                        for ci in range(len(LCH)):
                            emit_qkv(ci)
                            emit_v(ci)
                            if ci + 2 < len(LCH):
                                emit_ln(ci + 2, *emit_stats(ci + 2))

                # ---------------- P2: attention + proj ----------------
                with tc.tile_pool(name="wpp", bufs=1) as wpp, \
                     tc.tile_pool(name="qbp", bufs=1) as qbp, \
                     tc.tile_pool(name="oal", bufs=2) as oal, \
                     tc.tile_pool(name="p2t", bufs=2) as p2t, \
                     tc.tile_pool(name="pexp", bufs=2) as pexp, \
                     tc.tile_pool(name="psA", bufs=1, space="PSUM") as psA, \
                     tc.tile_pool(name="psB", bufs=1, space="PSUM") as psB, \
                     tc.tile_pool(name="psC", bufs=1, space="PSUM") as psC, \
                     tc.tile_pool(name="psS", bufs=1, space="PSUM") as psS:
                    wp_sb = [wpp.tile([128, C], BF16, name=f"wp{j}")
                             for j in range(6)]
                    qbbuf = [qbp.tile([128, N], BF16, name=f"qb{j}")
                             for j in range(4)]
                    for j in range(4):
                        p0 = (j % 2) * 64
                        nc.vector.memset(qbbuf[j][64 - p0:128 - p0, :], 0.0)

                    seq = [(b, h) for b in range(BPC) for h in range(H)]
                    o_tiles = {}
                    estate = {}

                    def qb_dma(idx):
                        b, h = seq[idx]
                        base = b * N
                        p0 = (h % 2) * 64
                        nc.sync.dma_start(
                            out=qbbuf[h % 4][p0:p0 + 64, :],
                            in_=qk_sb[h // 2][p0:p0 + 64, base:base + N])

                    def front(idx):
                        b, h = seq[idx]
                        if idx + 2 < len(seq):
                            qb_dma(idx + 2)
                        base = b * N
                        qb = qbbuf[h % 4]
                        kt = qk_sb[6 + h // 2]
                        quad = psA.tile([128, 2048], F32, name=f"qd{b}_{h}",
                                        tag="quad")
                        for i in range(4):
                            nc.tensor.matmul(
                                quad[:, i * 512:(i + 1) * 512],
                                kt[:, base + i * 128:base + (i + 1) * 128],
                                qb[:, 0:512], start=True, stop=True)
                        eA = pexp.tile([128, 2048], BF16, name=f"eA{b}_{h}",
                                       tag="eA")
                        nc.scalar.activation(eA[:], quad[:], AF.Exp)
                        bt = psB.tile([128, 837], F32, name=f"bt{b}_{h}",
                                      tag="sb")
                        nc.tensor.matmul(bt[:, 0:512],
                                         kt[:, base + 512:base + 640],
                                         qb[:, 0:512], start=True, stop=True)
                        for i in range(5):
                            ks = kt[:, base + i * 128:base + i * 128 + 128]
                            nc.tensor.matmul(
                                bt[:, 512 + i * 65:512 + (i + 1) * 65],
                                ks, qb[:, 512:577], start=True, stop=True)
                        eB = pexp.tile([128, 837], BF16, name=f"eB{b}_{h}",
                                       tag="eB")
                        nc.scalar.activation(eB[:], bt[:], AF.Exp)
                        estate[idx] = (eA, eB)

                    def mid(idx):
                        b, h = seq[idx]
                        eA, eB = estate.pop(idx)
                        pso = psC.tile([65, 512], F32, name=f"po{b}_{h}",
                                       tag="pso")
                        for i in range(5):
                            kr = KTILES[i][1]
                            mov = (eA[:, i * 512:(i + 1) * 512] if i < 4
                                   else eB[:, 0:512])
                            nc.tensor.matmul(pso[:],
                                             vbuf[(b, i)][:kr,
                                                          h * 65:(h + 1) * 65],
                                             mov[:kr, :],
                                             start=(i == 0), stop=(i == 4))
                        avt = psS.tile([65, 65], F32, name=f"pt{b}_{h}",
                                       tag="sm")
                        for i in range(5):
                            kr = KTILES[i][1]
                            nc.tensor.matmul(
                                avt[:],
                                vbuf[(b, i)][:kr, h * 65:(h + 1) * 65],
                                eB[:kr, 512 + i * 65:512 + (i + 1) * 65],
                                start=(i == 0), stop=(i == 4))
                        oh = p2t.tile([65, N], BF16, name=f"oh{b}_{h}",
                                      tag=f"oh{h % 2}")
                        nc.vector.tensor_copy(oh[:, 0:512], pso[:])
                        nc.vector.tensor_copy(oh[:, 512:577], avt[:])
                        den = p2t.tile([1, N], F32, name=f"dn{b}_{h}",
                                       tag="den")
                        nc.vector.tensor_copy(den[:], oh[64:65, :])
                        rcp = p2t.tile([1, N], F32, name=f"rc{b}_{h}",
                                       tag="rcp")
                        nc.vector.reciprocal_approx_fast(rcp[:], den[:])
                        bc = p2t.tile([64, N], F32, name=f"bc{b}_{h}",
                                      tag=f"bc{h % 2}")
                        nc.gpsimd.partition_broadcast(bc[:], rcp[0:1, :])
                        return oh, bc

                    ostate = {}

                    def tail(idx):
                        b, h = seq[idx]
                        oh, bc = ostate.pop(idx)
                        p0 = (h % 2) * 64
                        nc.vector.tensor_tensor(
                            o_tiles[b][h // 2][p0:p0 + 64, :],
                            oh[0:64, :], bc[:], ALU.mult)

                    def emit_proj_n(b, n):
                        base = b * N
                        ppq = psS.tile([128, 512], F32, name=f"pp{b}_{n}",
                                       tag="sm")
                        ppt = psS.tile([128, 65], F32, name=f"pq{b}_{n}",
                                       tag="sm")
                        for j in range(6):
                            nc.tensor.matmul(
                                ppq[:], wp_sb[j][:, n * 128:(n + 1) * 128],
                                o_tiles[b][j][:, 0:512],
                                start=(j == 0), stop=(j == 5))
                        for j in range(6):
                            nc.tensor.matmul(
                                ppt[:], wp_sb[j][:, n * 128:(n + 1) * 128],
                                o_tiles[b][j][:, 512:577],
                                start=(j == 0), stop=(j == 5))
                        xr = p2t.tile([128, N], BF16, name=f"xr{b}_{n}",
                                      tag=f"xr{n % 2}")
                        nc.sync.dma_start(
                            out=xr[:],
                            in_=xT_d[n * 128:(n + 1) * 128, base:base + N])
                        nc.vector.scalar_tensor_tensor(
                            x2[n][:, base:base + 512], ppq[:],
                            pb[:, n:n + 1], xr[:, 0:512], ALU.add, ALU.add)
                        nc.vector.scalar_tensor_tensor(
                            x2[n][:, base + 512:base + N], ppt[:],
                            pb[:, n:n + 1], xr[:, 512:577], ALU.add, ALU.add)

                    qb_dma(0)
                    qb_dma(1)
                    for j in range(6):
                        nc.scalar.dma_start(out=wp_sb[j][:], in_=wp_d[j])
                    for idx, (b, h) in enumerate(seq):
                        if h == 0:
                            o_tiles[b] = [oal.tile([128, N], BF16,
                                                   name=f"oa{b}_{j}",
                                                   tag=f"oa{j}")
                                          for j in range(6)]
                        front(idx)
                        if idx >= 1:
                            ostate[idx - 1] = mid(idx - 1)
                        if idx >= 3:
                            tail(idx - 3)
                        if b > 0 and 2 <= h < 8:
                            emit_proj_n(b - 1, h - 2)
                    ostate[len(seq) - 1] = mid(len(seq) - 1)
                    tail(len(seq) - 3)
                    tail(len(seq) - 2)
                    tail(len(seq) - 1)
                    for n in range(6):
                        emit_proj_n(BPC - 1, n)

            # ---------------- P4: LN2 + MLP ----------------
            with tc.tile_pool(name="w12", bufs=1) as w12, \
                 tc.tile_pool(name="p4x", bufs=2) as p4x, \
                 tc.tile_pool(name="p4s", bufs=2) as p4s, \
                 tc.tile_pool(name="h2p", bufs=2) as h2p, \
                 tc.tile_pool(name="p4t", bufs=2) as p4t, \
                 tc.tile_pool(name="pgl", bufs=2) as pgl, \
                 tc.tile_pool(name="ps4s", bufs=1, space="PSUM") as ps4s, \
                 tc.tile_pool(name="ps41", bufs=4, space="PSUM") as ps41, \
                 tc.tile_pool(name="ps42", bufs=2, space="PSUM") as ps42:
                w1_sb = [w12.tile([128, 2 * HID], F8, name=f"w1_{k}")
                         for k in range(3)]
                for k in range(3):
                    nc.sync.dma_start(out=w1_sb[k][:], in_=w1_d[k])
                w2_sb = [w12.tile([128, 2 * C], F8, name=f"w2_{k}")
                         for k in range(12)]
                h2c = {}

                def emit_stats2(ci):
                    c0, cw = CHUNKS[ci]
                    ps_sum = ps4s.tile([128, cw], F32, name=f"2ss_{c0}",
                                       tag="ps_sum2")
                    ps_ssq = ps4s.tile([128, cw], F32, name=f"2sq_{c0}",
                                       tag="ps_ssq2")
                    for k in range(6):
                        xq = p4x.tile([128, cw], BF16, name=f"2xq{k}_{c0}",
                                      tag=f"2xq{k}")
                        nc.vector.tensor_tensor(xq[:], x2[k][:, c0:c0 + cw],
                                                x2[k][:, c0:c0 + cw], ALU.mult)
                        nc.tensor.matmul(ps_sum[:], ones128[:],
                                         x2[k][:, c0:c0 + cw],
                                         start=(k == 0), stop=(k == 5))
                        nc.tensor.matmul(ps_ssq[:], ones128[:], xq[:],
                                         start=(k == 0), stop=(k == 5))
                    mu = p4s.tile([128, cw], F32, name=f"2mu_{c0}", tag="2mu")
                    nc.vector.tensor_scalar(mu[:], ps_sum[:], 1.0 / C, None,
                                            ALU.mult)
                    t1 = p4s.tile([128, cw], F32, name=f"2t1_{c0}", tag="2tt")
                    nc.vector.tensor_tensor(t1[:], ps_sum[:], mu[:], ALU.mult)
                    t2 = p4s.tile([128, cw], F32, name=f"2t2_{c0}", tag="2tt")
                    nc.vector.tensor_tensor(t2[:], ps_ssq[:], t1[:],
                                            ALU.subtract)
                    t3 = p4s.tile([128, cw], F32, name=f"2t3_{c0}", tag="2tt")
                    nc.vector.tensor_scalar(t3[:], t2[:], 1.0 / C, 1e-5,
                                            ALU.mult, ALU.add)
                    iv = p4s.tile([128, cw], F32, name=f"2iv_{c0}", tag="2iv")
                    nc.vector.reciprocal_approx_fast(iv[:], t3[:])
                    rs = p4s.tile([128, cw], BF16, name=f"2rs_{c0}", tag="2rs")
                    nc.scalar.activation(rs[:], iv[:], AF.Sqrt)
                    murs = p4s.tile([128, cw], BF16, name=f"2mr_{c0}",
                                    tag="2mr")
                    nc.vector.tensor_tensor(murs[:], mu[:], rs[:], ALU.mult)
                    hs = []
                    for i in range(3):
                        hp = h2p.tile([128, 2 * cw], F8, name=f"2h{i}_{c0}",
                                      tag=f"2h{i}")
                        hs.append(hp)
                    for k in range(6):
                        tmp = p4s.tile([128, cw], F32, name=f"2tp{k}_{c0}",
                                       tag="2tp")
                        nc.vector.tensor_tensor(tmp[:], x2[k][:, c0:c0 + cw],
                                                rs[:], ALU.mult)
                        dst = hs[k // 2][:, (k % 2) * cw:(k % 2) * cw + cw]
                        with nc.allow_low_precision(reason="fp8 mlp act"):
                            nc.vector.tensor_tensor(dst, tmp[:], murs[:],
                                                    ALU.subtract)
                    h2c[ci] = hs

                P4CH = [0, 1, 2, 4, 3]
                emit_stats2(P4CH[0])
                for pi in range(5):
                    ci = P4CH[pi]
                    c0, cw = CHUNKS[ci]
                    hs = h2c.pop(ci)
                    gl = [pgl.tile([128, 2 * cw], F8, name=f"gl{j}_{c0}",
                                   tag=f"gl{j}") for j in range(12)]
                    for n1 in range(24):
                        p1p = ps41.tile([128, cw], F32, name=f"p41_{n1}_{c0}",
                                        tag="ps41")
                        for i in range(3):
                            w3 = w1_sb[i].rearrange(
                                "p (s n) -> p s n", s=2)[
                                :, :, n1 * 128:(n1 + 1) * 128]
                            h3 = hs[i].rearrange("p (s f) -> p s f", s=2)
                            nc.tensor.matmul(p1p[:], w3, h3,
                                             start=(i == 0), stop=(i == 2),
                                             perf_mode=DR)
                        gdst = gl[n1 // 2][:, (n1 % 2) * cw:(n1 % 2) * cw + cw]
                        with nc.allow_low_precision(reason="fp8 mlp act"):
                            nc.scalar.activation(gdst, p1p[:], AF.Gelu,
                                                 bias=b1a[:, n1:n1 + 1],
                                                 scale=1.0 / WS)
                    if pi == 0:
                        for k in range(12):
                            nc.sync.dma_start(out=w2_sb[k][:], in_=w2_d[k])
                    if pi + 1 < 5:
                        emit_stats2(P4CH[pi + 1])
                    for n2 in range(6):
                        p2p = ps42.tile([128, cw], F32, name=f"p42_{n2}_{c0}",
                                        tag="ps42")
                        for j in range(12):
                            w3 = w2_sb[j].rearrange(
                                "p (s n) -> p s n", s=2)[
                                :, :, n2 * 128:(n2 + 1) * 128]
                            g3 = gl[j].rearrange("p (s f) -> p s f", s=2)
                            nc.tensor.matmul(p2p[:], w3, g3,
                                             start=(j == 0), stop=(j == 11),
                                             perf_mode=DR)
                        t2o = p4t.tile([128, cw], F32, name=f"t2o{n2}_{c0}",
                                       tag="t2o")
                        nc.scalar.activation(t2o[:], p2p[:], AF.Identity,
                                             bias=b2a[:, n2:n2 + 1],
                                             scale=1.0 / WS)
                        oo = p4t.tile([128, cw], F32, name=f"oo{n2}_{c0}",
                                      tag="oo")
                        nc.vector.tensor_tensor(oo[:], t2o[:],
                                                x2[n2][:, c0:c0 + cw], ALU.add)
                        nc.sync.dma_start(
                            out=outT_d[n2 * 128:(n2 + 1) * 128, c0:c0 + cw],
                            in_=oo[:])
    nc.compile()
    return nc


_CACHE = {}


def _prep_shared(inputs):
    f32 = np.float32
    bf = ml_dtypes.bfloat16
    qkv_w = np.asarray(inputs["qkv_w"], f32)
    ln1_g = np.asarray(inputs["ln1_g"], f32)
    ln1_b = np.asarray(inputs["ln1_b"], f32)
    qkv_b = np.asarray(inputs["qkv_b"], f32)
    W = qkv_w * ln1_g[:, None]
    bq = ln1_b @ qkv_w + qkv_b
    W = W.copy()
    W[:, :C] *= 0.125
    bq = bq.copy()
    bq[:C] *= 0.125

    proj_w = np.asarray(inputs["proj_w"], f32)
    fc1_w = np.asarray(inputs["fc1_w"], f32)
    ln2_g = np.asarray(inputs["ln2_g"], f32)
    ln2_b = np.asarray(inputs["ln2_b"], f32)
    fc1_b = np.asarray(inputs["fc1_b"], f32)
    W1 = fc1_w * ln2_g[:, None]
    b1 = ln2_b @ fc1_w + fc1_b
    fc2_w = np.asarray(inputs["fc2_w"], f32)

    f8 = ml_dtypes.float8_e4m3
    w1s = (W1 * WS).astype(f8).reshape(3, 2, 128, HID)
    w1s = w1s.transpose(0, 2, 1, 3).reshape(3, 128, 2 * HID)
    w2s = (fc2_w * WS).astype(f8).reshape(12, 2, 128, C)
    w2s = w2s.transpose(0, 2, 1, 3).reshape(12, 128, 2 * C)
    return {
        "wqkv": np.ascontiguousarray(W.astype(bf)),
        "qkb": np.ascontiguousarray(bq[:2 * C].reshape(12, 128).T.astype(f32)),
        "vbb": np.ascontiguousarray(np.tile(bq[2 * C:], (128, 1)).astype(f32)),
        "wp": np.ascontiguousarray(proj_w.reshape(6, 128, C).astype(bf)),
        "pb": np.ascontiguousarray(
            np.asarray(inputs["proj_b"], f32).reshape(6, 128).T),
        "w1": np.ascontiguousarray(w1s),
        "b1a": np.ascontiguousarray(b1.reshape(24, 128).T.astype(f32)),
        "w2": np.ascontiguousarray(w2s),
        "b2a": np.ascontiguousarray(
            np.asarray(inputs["fc2_b"], f32).reshape(6, 128).T),
    }


def _make_in_maps(inputs):
    bf = ml_dtypes.bfloat16
    x = np.asarray(inputs["x"], np.float32)
    shared = _prep_shared(inputs)
    in_maps = []
    for c in range(NCORES):
        xT = np.ascontiguousarray(
            x[c * BPC:(c + 1) * BPC].reshape(T, C).T.astype(bf))
        m = {"xT": xT}
        m.update(shared)
        in_maps.append(m)
    return in_maps


def kernel(**inputs):
    if "nc" not in _CACHE:
        _CACHE["nc"] = _build_nc()
    nc = _CACHE["nc"]
    in_maps = _make_in_maps(inputs)
    res = run_bass_kernel_spmd(nc, in_maps, list(range(NCORES)))
    out = np.empty((B, N, C), np.float32)
    for c in range(NCORES):
        outT = res.results[c]["outT"]
        out[c * BPC:(c + 1) * BPC] = outT.T.reshape(BPC, N, C)
    return out


# revision 39
# speedup vs baseline: 1.0321x; 1.0062x over previous
"""Fused transformer block (pre-norm attn + MLP) for Trainium2, 8 cores.

Sharding: data-parallel over batch (32 batches -> 4 per core), no
collectives. Each core computes the full block on its shard.

v3 design notes (vs v2 at 672us; this version ~532us):
- LN rsqrt via DVE reciprocal_approx_fast + one ACT Sqrt instead of
  Ln+Exp: Ln/Exp live in different ACT table-sets, so v2 paid 2 table
  loads (~2.6us each) per LN chunk (31 loads total; now ~10).
- P1 processes the token stream in global chunks (128,384,512x3,260)
  instead of per-batch (512,65): the 65-wide qkv matmuls were
  LDWEIGHTS-bound (107ns for 27ns of stream). LN1 output h lives in
  persistent [128, T] tiles so per-batch v k-tiles can slice any token
  range regardless of chunk boundaries. qkv weights stream in 3 column
  groups (q, k, v) at P1 top so the first qkv matmuls start early.
  All qkv psum evacs ride ACT Identity (ACT is ~20% busy in P1; the
  LN chains keep DVE at ~65%, and evacs on DVE stalled the psum ring).
- P2 exp runs as 2 ACT instructions per head instead of 5: scores for
  k-tiles 0-3 x queries 0-511 land in one [128, 2048] psum quad (4
  banks), everything else (k-tile 4 + the 65-query tails of all 5
  k-tiles) in one [128, 837] tile. ACT overhead is ~352 cols per
  instruction, so 5 -> 2 instructions saves ~0.9us/head (~42us).
  The kt4 stationary is padded to 128 columns (qk_sb over-allocated by
  64 junk cols, memset 0) so the B-tile psum is fully written and the
  single exp never reads unwritten psum.
- q zero-pad staging buffers are filled by SBUF->SBUF DMA (idle engine)
  instead of ACT Identity (v2: 48 x 640ns on the P2-critical ACT).
- softmax denominators: per-head [1,577] reciprocal_approx_fast (DVE
  custom op, ~5x faster than the iterative reciprocal), result already
  on partition 0 so gpsimd partition_broadcast needs no staging copies.
  The o-normalize multiply trails its head by 3 iterations so the
  gpsimd broadcast latency hides. (Measured regressions to avoid:
  gpsimd general tensor ops ~2x-10x slower than DVE; psum-tag sharing
  between PE streams and DVE-gated slots; ACT-side evacs; f32 oh with
  direct-recip -> NaN. Keep mult/den/evac on DVE, pools separate.)
- emission order per head: front (scores+exps) BEFORE the previous
  head's attn@v, so the PE never head-of-line blocks on ACT exp.
- proj is emitted one n-tile per head during the NEXT batch's first 6
  heads; psum stays at 8 banks (quad 4 + B 2 + av-q0 1 + small-shared 1
  for av-tail/proj tiles).
LN gains/biases and the attention scale are folded into the weights on
the host. All matmul operands bf16; psum f32; MLP weights fp8+DoubleRow.
ACT exp has no max subtraction (|scores| < 3).
"""
import numpy as np
import ml_dtypes
import concourse.bacc as bacc
import concourse.mybir as mybir
import concourse.tile as tile
from concourse.bass_utils import run_bass_kernel_spmd

F32 = mybir.dt.float32
BF16 = mybir.dt.bfloat16
F8 = mybir.dt.float8e4
DR = mybir.MatmulPerfMode.DoubleRow
WS = 16.0  # fp8 weight scale (w*WS stored fp8; ACT evac rescales by 1/WS)
AF = mybir.ActivationFunctionType
ALU = mybir.AluOpType

B, N, C = 32, 577, 768
H, D = 12, 64
HID = 3072
NCORES = 8
BPC = B // NCORES            # 4 batches per core
T = BPC * N                  # 2308 tokens per core
TP = T + 64                  # qk_sb padded so kt4 stationaries are 128 wide
CHUNKS = [(0, 512), (512, 512), (1024, 512), (1536, 512), (2048, 260)]
LCH = [(0, 256), (256, 256), (512, 512), (1024, 512), (1536, 512), (2048, 260)]
KTILES = [(0, 128), (128, 128), (256, 128), (384, 128), (512, 65)]
# v k-tiles flushed after the LN chunk that completes them:
# (batch, i, abs_start, rows) grouped by first chunk index that covers them
VFLUSH = {ci: [] for ci in range(len(LCH))}
for _b in range(BPC):
    for _i, (_k0, _kr) in enumerate(KTILES):
        _end = _b * N + _k0 + _kr
        for _ci, (_c0, _cw) in enumerate(LCH):
            if _end <= _c0 + _cw:
                VFLUSH[_ci].append((_b, _i, _b * N + _k0, _kr))
                break


def _build_nc():
    nc = bacc.Bacc("TRN2", target_bir_lowering=False, debug=False,
                   num_devices=NCORES)
    xT_d = nc.dram_tensor("xT", [C, T], BF16, kind="ExternalInput")
    x8_d = nc.dram_tensor("x8", [3, 128, 2 * T], F8, kind="ExternalInput")
    wqkv_d = nc.dram_tensor("wqkv", [C, 3 * C], BF16, kind="ExternalInput")
    qkb_d = nc.dram_tensor("qkb", [128, 12], F32, kind="ExternalInput")
    vbb_d = nc.dram_tensor("vbb", [128, C], BF16, kind="ExternalInput")
    wp_d = nc.dram_tensor("wp", [6, 128, C], BF16, kind="ExternalInput")
    pb_d = nc.dram_tensor("pb", [128, 6], F32, kind="ExternalInput")
    w1_d = nc.dram_tensor("w1", [3, 128, 2 * HID], F8, kind="ExternalInput")
    b1a_d = nc.dram_tensor("b1a", [128, 24], F32, kind="ExternalInput")
    w2_d = nc.dram_tensor("w2", [12, 128, 2 * C], F8, kind="ExternalInput")
    b2a_d = nc.dram_tensor("b2a", [128, 6], F32, kind="ExternalInput")
    outT_d = nc.dram_tensor("outT", [C, T], F32, kind="ExternalOutput")

    with tile.TileContext(nc) as tc:
        with tc.tile_pool(name="cst", bufs=1) as cst, \
             tc.tile_pool(name="x2p", bufs=1) as x2p:
            ones128 = cst.tile([128, 128], BF16)
            nc.vector.memset(ones128[:], 1.0)
            ones8 = cst.tile([128, 256], F8)
            nc.vector.memset(ones8[:], 1.0)
            ones8r = ones8.rearrange("p (s n) -> p s n", s=2)
            qkb = cst.tile([128, 12], F32)
            nc.sync.dma_start(out=qkb[:], in_=qkb_d[:])
            vbb = cst.tile([128, C], BF16)
            nc.sync.dma_start(out=vbb[:], in_=vbb_d[:])
            pb = cst.tile([128, 6], F32)
            nc.sync.dma_start(out=pb[:], in_=pb_d[:])
            b1a = cst.tile([128, 24], F32)
            nc.sync.dma_start(out=b1a[:], in_=b1a_d[:])
            b2a = cst.tile([128, 6], F32)
            nc.sync.dma_start(out=b2a[:], in_=b2a_d[:])
            x2 = [x2p.tile([128, T], BF16, name=f"x2_{k}") for k in range(6)]

            with tc.tile_pool(name="qks", bufs=1) as qks, \
                 tc.tile_pool(name="vbp", bufs=1) as vbp:
                qk_sb = [qks.tile([128, TP], BF16, name=f"qk{n}")
                         for n in range(12)]
                for n in range(6, 12):
                    nc.vector.memset(qk_sb[n][:, T:TP], 0.0)
                vbuf = {}
                for b in range(BPC):
                    for i in range(5):
                        vbuf[(b, i)] = vbp.tile([128, H * 65], BF16,
                                                name=f"vb{b}_{i}")
                        ocol = vbuf[(b, i)].rearrange(
                            "p (h e) -> p h e", e=65)[:, :, 64]
                        nc.vector.memset(ocol, 1.0)

                # ---------------- P1: LN1 + qk + v ----------------
                with tc.tile_pool(name="p1w", bufs=1) as p1w, \
                     tc.tile_pool(name="p1hh", bufs=1) as p1hh:
                    wq = [p1w.tile([128, 3 * C], BF16, name=f"wq{k}")
                          for k in range(6)]
                    h_all = [p1hh.tile([128, T], BF16, name=f"ha{k}")
                             for k in range(6)]
                    with tc.tile_pool(name="p1x", bufs=2) as p1x, \
                         tc.tile_pool(name="p1s", bufs=2) as p1s, \
                         tc.tile_pool(name="ps1", bufs=1, space="PSUM") as ps1, \
                         tc.tile_pool(name="psqk", bufs=2, space="PSUM") as psqk, \
                         tc.tile_pool(name="psv", bufs=2, space="PSUM") as psv:
                        xc = {}

                        def emit_stats(ci):
                            c0, cw = LCH[ci]
                            xs = []
                            for k in range(6):
                                xt = p1x.tile([128, cw], BF16,
                                              name=f"x{k}_{c0}", tag=f"x{k}")
                                nc.sync.dma_start(
                                    out=xt[:],
                                    in_=xT_d[k * 128:(k + 1) * 128,
                                             c0:c0 + cw])
                                xs.append(xt)
                            xc[ci] = xs
                            ps_sum = ps1.tile([128, cw], F32,
                                              name=f"pss_{c0}", tag="ps_sum")
                            ps_ssq = ps1.tile([128, cw], F32,
                                              name=f"psq_{c0}", tag="ps_ssq")
                            for i in range(3):
                                x8 = p1x.tile([128, 2 * cw], F8,
                                              name=f"x8{i}_{c0}", tag=f"x8{i}")
                                nc.sync.dma_start(
                                    out=x8[:],
                                    in_=x8_d[i][:, 2 * c0:2 * (c0 + cw)])
                                xq = p1x.tile([128, 2 * cw], F8,
                                              name=f"xq{i}_{c0}", tag="xq")
                                with nc.allow_low_precision(
                                        reason="fp8 ln stats"):
                                    nc.vector.tensor_tensor(
                                        xq[:, 0:cw], xs[2 * i][:],
                                        xs[2 * i][:], ALU.mult)
                                    nc.vector.tensor_tensor(
                                        xq[:, cw:2 * cw], xs[2 * i + 1][:],
                                        xs[2 * i + 1][:], ALU.mult)
                                x8r = x8.rearrange("p (s f) -> p s f", s=2)
                                xqr = xq.rearrange("p (s f) -> p s f", s=2)
                                nc.tensor.matmul(ps_sum[:], ones8r, x8r,
                                                 start=(i == 0), stop=(i == 2),
                                                 perf_mode=DR)
                                nc.tensor.matmul(ps_ssq[:], ones8r, xqr,
                                                 start=(i == 0), stop=(i == 2),
                                                 perf_mode=DR)
                            return ps_sum, ps_ssq

                        def emit_ln(ci, ps_sum, ps_ssq):
                            c0, cw = LCH[ci]
                            mu = p1s.tile([128, cw], BF16, name=f"mu_{c0}",
                                          tag="mu")
                            nc.vector.tensor_scalar(mu[:], ps_sum[:], 1.0 / C,
                                                    None, ALU.mult)
                            t1 = p1s.tile([128, cw], F32, name=f"t1_{c0}",
                                          tag="tt")
                            nc.vector.tensor_tensor(t1[:], ps_sum[:], mu[:],
                                                    ALU.mult)
                            t2 = p1s.tile([128, cw], F32, name=f"t2_{c0}",
                                          tag="tt")
                            nc.vector.tensor_tensor(t2[:], ps_ssq[:], t1[:],
                                                    ALU.subtract)
                            t3 = p1s.tile([128, cw], F32, name=f"t3_{c0}",
                                          tag="tt")
                            nc.vector.tensor_scalar(t3[:], t2[:], 1.0 / C,
                                                    1e-5, ALU.mult, ALU.add)
                            iv = p1s.tile([128, cw], F32, name=f"iv_{c0}",
                                          tag="iv")
                            nc.vector.reciprocal_approx_fast(iv[:], t3[:])
                            rs = p1s.tile([128, cw], BF16, name=f"rs_{c0}",
                                          tag="rs")
                            nc.scalar.activation(rs[:], iv[:], AF.Sqrt)
                            murs = p1s.tile([128, cw], BF16, name=f"mr_{c0}",
                                            tag="mr")
                            nc.vector.tensor_tensor(murs[:], mu[:], rs[:],
                                                    ALU.mult)
                            for k in range(6):
                                tmp = p1s.tile([128, cw], BF16,
                                               name=f"tp{k}_{c0}", tag="tp")
                                nc.vector.tensor_tensor(tmp[:], xc[ci][k][:],
                                                        rs[:], ALU.mult)
                                nc.vector.tensor_tensor(
                                    h_all[k][:, c0:c0 + cw], tmp[:], murs[:],
                                    ALU.subtract)
                            del xc[ci]

                        def emit_qkv(ci):
                            c0, cw = LCH[ci]
                            for n in range(12):
                                pq = psqk.tile([128, cw], F32,
                                               name=f"pq{n}_{c0}", tag="psqk")
                                for k in range(6):
                                    nc.tensor.matmul(
                                        pq[:], wq[k][:, n * 128:(n + 1) * 128],
                                        h_all[k][:, c0:c0 + cw],
                                        start=(k == 0), stop=(k == 5))
                                nc.scalar.activation(
                                    qk_sb[n][:, c0:c0 + cw], pq[:],
                                    AF.Identity, bias=qkb[:, n:n + 1])

                        def emit_v(ci):
                            for (b, i, a0, kr) in VFLUSH[ci]:
                                pv = psv.tile([128, C], F32,
                                              name=f"pv_{b}_{i}", tag="psv")
                                for k in range(6):
                                    nc.tensor.matmul(
                                        pv[:kr, 0:512],
                                        h_all[k][:, a0:a0 + kr],
                                        wq[k][:, 1536:2048],
                                        start=(k == 0), stop=(k == 5))
                                for k in range(6):
                                    nc.tensor.matmul(
                                        pv[:kr, 512:768],
                                        h_all[k][:, a0:a0 + kr],
                                        wq[k][:, 2048:2304],
                                        start=(k == 0), stop=(k == 5))
                                vm = vbuf[(b, i)]
                                dst = vm[0:kr].rearrange(
                                    "p (h e) -> p h e", e=65)[:, :, 0:64]
                                srcv = pv[0:kr].rearrange(
                                    "p (h e) -> p h e", e=64)
                                bia = vbb[0:kr].rearrange(
                                    "p (h e) -> p h e", e=64)
                                nc.vector.tensor_tensor(dst, srcv, bia,
                                                        ALU.add)

                        for g0, g1 in ((0, 768), (768, 1536), (1536, 2304)):
                            for k in range(6):
                                nc.scalar.dma_start(
                                    out=wq[k][:, g0:g1],
                                    in_=wqkv_d[k * 128:(k + 1) * 128, g0:g1])
                        emit_ln(0, *emit_stats(0))
                        emit_ln(1, *emit_stats(1))
                        for ci in range(len(LCH)):
                            emit_qkv(ci)
                            emit_v(ci)
                            if ci + 2 < len(LCH):
                                emit_ln(ci + 2, *emit_stats(ci + 2))

                # ---------------- P2: attention + proj ----------------
                with tc.tile_pool(name="wpp", bufs=1) as wpp, \
                     tc.tile_pool(name="qbp", bufs=1) as qbp, \
                     tc.tile_pool(name="oal", bufs=2) as oal, \
                     tc.tile_pool(name="p2t", bufs=2) as p2t, \
                     tc.tile_pool(name="pexp", bufs=2) as pexp, \
                     tc.tile_pool(name="psA", bufs=1, space="PSUM") as psA, \
                     tc.tile_pool(name="psB", bufs=1, space="PSUM") as psB, \
                     tc.tile_pool(name="psC", bufs=1, space="PSUM") as psC, \
                     tc.tile_pool(name="psS", bufs=1, space="PSUM") as psS:
                    wp_sb = [wpp.tile([128, C], BF16, name=f"wp{j}")
                             for j in range(6)]
                    qbbuf = [qbp.tile([128, N], BF16, name=f"qb{j}")
                             for j in range(4)]
                    for j in range(4):
                        p0 = (j % 2) * 64
                        nc.vector.memset(qbbuf[j][64 - p0:128 - p0, :], 0.0)

                    seq = [(b, h) for b in range(BPC) for h in range(H)]
                    o_tiles = {}
                    estate = {}

                    def qb_dma(idx):
                        b, h = seq[idx]
                        base = b * N
                        p0 = (h % 2) * 64
                        nc.sync.dma_start(
                            out=qbbuf[h % 4][p0:p0 + 64, :],
                            in_=qk_sb[h // 2][p0:p0 + 64, base:base + N])

                    def front(idx):
                        b, h = seq[idx]
                        if idx + 2 < len(seq):
                            qb_dma(idx + 2)
                        base = b * N
                        qb = qbbuf[h % 4]
                        kt = qk_sb[6 + h // 2]
                        quad = psA.tile([128, 2048], F32, name=f"qd{b}_{h}",
                                        tag="quad")
                        for i in range(4):
                            nc.tensor.matmul(
                                quad[:, i * 512:(i + 1) * 512],
                                kt[:, base + i * 128:base + (i + 1) * 128],
                                qb[:, 0:512], start=True, stop=True)
                        eA = pexp.tile([128, 2048], BF16, name=f"eA{b}_{h}",
                                       tag="eA")
                        nc.scalar.activation(eA[:], quad[:], AF.Exp)
                        bt = psB.tile([128, 837], F32, name=f"bt{b}_{h}",
                                      tag="sb")
                        nc.tensor.matmul(bt[:, 0:512],
                                         kt[:, base + 512:base + 640],
                                         qb[:, 0:512], start=True, stop=True)
                        for i in range(5):
                            ks = kt[:, base + i * 128:base + i * 128 + 128]
                            nc.tensor.matmul(
                                bt[:, 512 + i * 65:512 + (i + 1) * 65],
                                ks, qb[:, 512:577], start=True, stop=True)
                        eB = pexp.tile([128, 837], BF16, name=f"eB{b}_{h}",
                                       tag="eB")
                        nc.scalar.activation(eB[:], bt[:], AF.Exp)
                        estate[idx] = (eA, eB)

                    def mid(idx):
                        b, h = seq[idx]
                        eA, eB = estate.pop(idx)
                        pso = psC.tile([65, 512], F32, name=f"po{b}_{h}",
                                       tag="pso")
                        for i in range(5):
                            kr = KTILES[i][1]
                            mov = (eA[:, i * 512:(i + 1) * 512] if i < 4
                                   else eB[:, 0:512])
                            nc.tensor.matmul(pso[:],
                                             vbuf[(b, i)][:kr,
                                                          h * 65:(h + 1) * 65],
                                             mov[:kr, :],
                                             start=(i == 0), stop=(i == 4))
                        avt = psS.tile([65, 65], F32, name=f"pt{b}_{h}",
                                       tag="sm")
                        for i in range(5):
                            kr = KTILES[i][1]
                            nc.tensor.matmul(
                                avt[:],
                                vbuf[(b, i)][:kr, h * 65:(h + 1) * 65],
                                eB[:kr, 512 + i * 65:512 + (i + 1) * 65],
                                start=(i == 0), stop=(i == 4))
                        oh = p2t.tile([65, N], BF16, name=f"oh{b}_{h}",
                                      tag=f"oh{h % 2}")
                        nc.vector.tensor_copy(oh[:, 0:512], pso[:])
                        nc.vector.tensor_copy(oh[:, 512:577], avt[:])
                        den = p2t.tile([1, N], F32, name=f"dn{b}_{h}",
                                       tag="den")
                        nc.vector.tensor_copy(den[:], oh[64:65, :])
                        rcp = p2t.tile([1, N], F32, name=f"rc{b}_{h}",
                                       tag="rcp")
                        nc.vector.reciprocal_approx_fast(rcp[:], den[:])
                        bc = p2t.tile([64, N], F32, name=f"bc{b}_{h}",
                                      tag=f"bc{h % 2}")
                        nc.gpsimd.partition_broadcast(bc[:], rcp[0:1, :])
                        return oh, bc

                    ostate = {}

                    def tail(idx):
                        b, h = seq[idx]
                        oh, bc = ostate.pop(idx)
                        p0 = (h % 2) * 64
                        nc.vector.tensor_tensor(
                            o_tiles[b][h // 2][p0:p0 + 64, :],
                            oh[0:64, :], bc[:], ALU.mult)

                    def emit_proj_n(b, n):
                        base = b * N
                        ppq = psS.tile([128, 512], F32, name=f"pp{b}_{n}",
                                       tag="sm")
                        ppt = psS.tile([128, 65], F32, name=f"pq{b}_{n}",
                                       tag="sm")
                        for j in range(6):
                            nc.tensor.matmul(
                                ppq[:], wp_sb[j][:, n * 128:(n + 1) * 128],
                                o_tiles[b][j][:, 0:512],
                                start=(j == 0), stop=(j == 5))
                        for j in range(6):
                            nc.tensor.matmul(
                                ppt[:], wp_sb[j][:, n * 128:(n + 1) * 128],
                                o_tiles[b][j][:, 512:577],
                                start=(j == 0), stop=(j == 5))
                        xr = p2t.tile([128, N], BF16, name=f"xr{b}_{n}",
                                      tag=f"xr{n % 2}")
                        nc.sync.dma_start(
                            out=xr[:],
                            in_=xT_d[n * 128:(n + 1) * 128, base:base + N])
                        nc.vector.scalar_tensor_tensor(
                            x2[n][:, base:base + 512], ppq[:],
                            pb[:, n:n + 1], xr[:, 0:512], ALU.add, ALU.add)
                        nc.vector.scalar_tensor_tensor(
                            x2[n][:, base + 512:base + N], ppt[:],
                            pb[:, n:n + 1], xr[:, 512:577], ALU.add, ALU.add)

                    qb_dma(0)
                    qb_dma(1)
                    for j in range(6):
                        nc.scalar.dma_start(out=wp_sb[j][:], in_=wp_d[j])
                    for idx, (b, h) in enumerate(seq):
                        if h == 0:
                            o_tiles[b] = [oal.tile([128, N], BF16,
                                                   name=f"oa{b}_{j}",
                                                   tag=f"oa{j}")
                                          for j in range(6)]
                        front(idx)
                        if idx >= 1:
                            ostate[idx - 1] = mid(idx - 1)
                        if idx >= 3:
                            tail(idx - 3)
                        if b > 0 and 2 <= h < 8:
                            emit_proj_n(b - 1, h - 2)
                    ostate[len(seq) - 1] = mid(len(seq) - 1)
                    tail(len(seq) - 3)
                    tail(len(seq) - 2)
                    tail(len(seq) - 1)
                    for n in range(6):
                        emit_proj_n(BPC - 1, n)

            # ---------------- P4: LN2 + MLP ----------------
            with tc.tile_pool(name="w12", bufs=1) as w12, \
                 tc.tile_pool(name="p4x", bufs=2) as p4x, \
                 tc.tile_pool(name="p4s", bufs=2) as p4s, \
                 tc.tile_pool(name="h2p", bufs=2) as h2p, \
                 tc.tile_pool(name="p4t", bufs=2) as p4t, \
                 tc.tile_pool(name="pgl", bufs=2) as pgl, \
                 tc.tile_pool(name="ps4s", bufs=1, space="PSUM") as ps4s, \
                 tc.tile_pool(name="ps41", bufs=4, space="PSUM") as ps41, \
                 tc.tile_pool(name="ps42", bufs=2, space="PSUM") as ps42:
                w1_sb = [w12.tile([128, 2 * HID], F8, name=f"w1_{k}")
                         for k in range(3)]
                for k in range(3):
                    nc.sync.dma_start(out=w1_sb[k][:], in_=w1_d[k])
                w2_sb = [w12.tile([128, 2 * C], F8, name=f"w2_{k}")
                         for k in range(12)]
                h2c = {}

                def emit_stats2(ci):
                    c0, cw = CHUNKS[ci]
                    ps_sum = ps4s.tile([128, cw], F32, name=f"2ss_{c0}",
                                       tag="ps_sum2")
                    ps_ssq = ps4s.tile([128, cw], F32, name=f"2sq_{c0}",
                                       tag="ps_ssq2")
                    for i in range(3):
                        xq = p4x.tile([128, 2 * cw], F8, name=f"2xq{i}_{c0}",
                                      tag=f"2xq{i}")
                        with nc.allow_low_precision(reason="fp8 ln stats"):
                            nc.vector.tensor_tensor(
                                xq[:, 0:cw], x2[2 * i][:, c0:c0 + cw],
                                x2[2 * i][:, c0:c0 + cw], ALU.mult)
                            nc.vector.tensor_tensor(
                                xq[:, cw:2 * cw], x2[2 * i + 1][:, c0:c0 + cw],
                                x2[2 * i + 1][:, c0:c0 + cw], ALU.mult)
                        xqr = xq.rearrange("p (s f) -> p s f", s=2)
                        nc.tensor.matmul(ps_ssq[:], ones8r, xqr,
                                         start=(i == 0), stop=(i == 2),
                                         perf_mode=DR)
                    for k in range(6):
                        nc.tensor.matmul(ps_sum[:], ones128[:],
                                         x2[k][:, c0:c0 + cw],
                                         start=(k == 0), stop=(k == 5))
                    mu = p4s.tile([128, cw], F32, name=f"2mu_{c0}", tag="2mu")
                    nc.vector.tensor_scalar(mu[:], ps_sum[:], 1.0 / C, None,
                                            ALU.mult)
                    t1 = p4s.tile([128, cw], F32, name=f"2t1_{c0}", tag="2tt")
                    nc.vector.tensor_tensor(t1[:], ps_sum[:], mu[:], ALU.mult)
                    t2 = p4s.tile([128, cw], F32, name=f"2t2_{c0}", tag="2tt")
                    nc.vector.tensor_tensor(t2[:], ps_ssq[:], t1[:],
                                            ALU.subtract)
                    t3 = p4s.tile([128, cw], F32, name=f"2t3_{c0}", tag="2tt")
                    nc.vector.tensor_scalar(t3[:], t2[:], 1.0 / C, 1e-5,
                                            ALU.mult, ALU.add)
                    iv = p4s.tile([128, cw], F32, name=f"2iv_{c0}", tag="2iv")
                    nc.vector.reciprocal_approx_fast(iv[:], t3[:])
                    rs = p4s.tile([128, cw], BF16, name=f"2rs_{c0}", tag="2rs")
                    nc.scalar.activation(rs[:], iv[:], AF.Sqrt)
                    murs = p4s.tile([128, cw], BF16, name=f"2mr_{c0}",
                                    tag="2mr")
                    nc.vector.tensor_tensor(murs[:], mu[:], rs[:], ALU.mult)
                    hs = []
                    for i in range(3):
                        hp = h2p.tile([128, 2 * cw], F8, name=f"2h{i}_{c0}",
                                      tag=f"2h{i}")
                        hs.append(hp)
                    for k in range(6):
                        tmp = p4s.tile([128, cw], F32, name=f"2tp{k}_{c0}",
                                       tag="2tp")
                        nc.vector.tensor_tensor(tmp[:], x2[k][:, c0:c0 + cw],
                                                rs[:], ALU.mult)
                        dst = hs[k // 2][:, (k % 2) * cw:(k % 2) * cw + cw]
                        with nc.allow_low_precision(reason="fp8 mlp act"):
                            nc.vector.tensor_tensor(dst, tmp[:], murs[:],
                                                    ALU.subtract)
                    h2c[ci] = hs

                P4CH = [0, 1, 2, 4, 3]
                emit_stats2(P4CH[0])
                for pi in range(5):
                    ci = P4CH[pi]
                    c0, cw = CHUNKS[ci]
                    hs = h2c.pop(ci)
                    gl = [pgl.tile([128, 2 * cw], F8, name=f"gl{j}_{c0}",
                                   tag=f"gl{j}") for j in range(12)]
                    for n1 in range(24):
                        p1p = ps41.tile([128, cw], F32, name=f"p41_{n1}_{c0}",
                                        tag="ps41")
                        for i in range(3):
                            w3 = w1_sb[i].rearrange(
                                "p (s n) -> p s n", s=2)[
                                :, :, n1 * 128:(n1 + 1) * 128]
                            h3 = hs[i].rearrange("p (s f) -> p s f", s=2)
                            nc.tensor.matmul(p1p[:], w3, h3,
                                             start=(i == 0), stop=(i == 2),
                                             perf_mode=DR)
                        gdst = gl[n1 // 2][:, (n1 % 2) * cw:(n1 % 2) * cw + cw]
                        with nc.allow_low_precision(reason="fp8 mlp act"):
                            nc.scalar.activation(gdst, p1p[:], AF.Gelu,
                                                 bias=b1a[:, n1:n1 + 1],
                                                 scale=1.0 / WS)
                    if pi == 0:
                        for k in range(12):
                            nc.sync.dma_start(out=w2_sb[k][:], in_=w2_d[k])
                    if pi + 1 < 5:
                        emit_stats2(P4CH[pi + 1])
                    for n2 in range(6):
                        p2p = ps42.tile([128, cw], F32, name=f"p42_{n2}_{c0}",
                                        tag="ps42")
                        for j in range(12):
                            w3 = w2_sb[j].rearrange(
                                "p (s n) -> p s n", s=2)[
                                :, :, n2 * 128:(n2 + 1) * 128]
                            g3 = gl[j].rearrange("p (s f) -> p s f", s=2)
                            nc.tensor.matmul(p2p[:], w3, g3,
                                             start=(j == 0), stop=(j == 11),
                                             perf_mode=DR)
                        t2o = p4t.tile([128, cw], F32, name=f"t2o{n2}_{c0}",
                                       tag="t2o")
                        nc.scalar.activation(t2o[:], p2p[:], AF.Identity,
                                             bias=b2a[:, n2:n2 + 1],
                                             scale=1.0 / WS)
                        oo = p4t.tile([128, cw], F32, name=f"oo{n2}_{c0}",
                                      tag="oo")
                        nc.vector.tensor_tensor(oo[:], t2o[:],
                                                x2[n2][:, c0:c0 + cw], ALU.add)
                        nc.sync.dma_start(
                            out=outT_d[n2 * 128:(n2 + 1) * 128, c0:c0 + cw],
                            in_=oo[:])
    nc.compile()
    return nc


_CACHE = {}


def _prep_shared(inputs):
    f32 = np.float32
    bf = ml_dtypes.bfloat16
    qkv_w = np.asarray(inputs["qkv_w"], f32)
    ln1_g = np.asarray(inputs["ln1_g"], f32)
    ln1_b = np.asarray(inputs["ln1_b"], f32)
    qkv_b = np.asarray(inputs["qkv_b"], f32)
    W = qkv_w * ln1_g[:, None]
    bq = ln1_b @ qkv_w + qkv_b
    W = W.copy()
    W[:, :C] *= 0.125
    bq = bq.copy()
    bq[:C] *= 0.125

    proj_w = np.asarray(inputs["proj_w"], f32)
    fc1_w = np.asarray(inputs["fc1_w"], f32)
    ln2_g = np.asarray(inputs["ln2_g"], f32)
    ln2_b = np.asarray(inputs["ln2_b"], f32)
    fc1_b = np.asarray(inputs["fc1_b"], f32)
    W1 = fc1_w * ln2_g[:, None]
    b1 = ln2_b @ fc1_w + fc1_b
    fc2_w = np.asarray(inputs["fc2_w"], f32)

    f8 = ml_dtypes.float8_e4m3
    w1s = (W1 * WS).astype(f8).reshape(3, 2, 128, HID)
    w1s = w1s.transpose(0, 2, 1, 3).reshape(3, 128, 2 * HID)
    w2s = (fc2_w * WS).astype(f8).reshape(12, 2, 128, C)
    w2s = w2s.transpose(0, 2, 1, 3).reshape(12, 128, 2 * C)
    return {
        "wqkv": np.ascontiguousarray(W.astype(bf)),
        "qkb": np.ascontiguousarray(bq[:2 * C].reshape(12, 128).T.astype(f32)),
        "vbb": np.ascontiguousarray(np.tile(bq[2 * C:], (128, 1)).astype(bf)),
        "wp": np.ascontiguousarray(proj_w.reshape(6, 128, C).astype(bf)),
        "pb": np.ascontiguousarray(
            np.asarray(inputs["proj_b"], f32).reshape(6, 128).T),
        "w1": np.ascontiguousarray(w1s),
        "b1a": np.ascontiguousarray(b1.reshape(24, 128).T.astype(f32)),
        "w2": np.ascontiguousarray(w2s),
        "b2a": np.ascontiguousarray(
            np.asarray(inputs["fc2_b"], f32).reshape(6, 128).T),
    }


def _make_in_maps(inputs):
    bf = ml_dtypes.bfloat16
    x = np.asarray(inputs["x"], np.float32)
    shared = _prep_shared(inputs)
    in_maps = []
    f8 = ml_dtypes.float8_e4m3
    for c in range(NCORES):
        xTf = x[c * BPC:(c + 1) * BPC].reshape(T, C).T
        xT = np.ascontiguousarray(xTf.astype(bf))
        x8 = xTf.astype(f8).reshape(3, 2, 128, T).transpose(0, 2, 1, 3)
        x8 = np.ascontiguousarray(x8.reshape(3, 128, 2 * T))
        m = {"xT": xT, "x8": x8}
        m.update(shared)
        in_maps.append(m)
    return in_maps


def kernel(**inputs):
    if "nc" not in _CACHE:
        _CACHE["nc"] = _build_nc()
    nc = _CACHE["nc"]
    in_maps = _make_in_maps(inputs)
    res = run_bass_kernel_spmd(nc, in_maps, list(range(NCORES)))
    out = np.empty((B, N, C), np.float32)
    for c in range(NCORES):
        outT = res.results[c]["outT"]
        out[c * BPC:(c + 1) * BPC] = outT.T.reshape(BPC, N, C)
    return out
